# revision 16
# baseline (speedup 1.0000x reference)
"""MoE-with-DeepGEMM kernel for 8 Trainium2 NeuronCores.

Problem: M=4096 tokens, D=2048 in-dim, H=2048 out-dim, E=8 experts.
    gate = softmax(x @ gate_w.T + gate_b)            # [M, E], fp32
    y    = (q8(x) @ q8(expert_w[e]).T) -> bf16       # [E, M, H]
    out  = sum_e gate[:, e, None] * y[e].astype(f32) # [M, H]

Strategy: data-parallel over tokens (M). Each of the 8 cores gets
M/8 = 512 tokens, all 8 experts' weights, and computes its output slice
independently - no collectives; the host concatenates the slices.

The PE floor is 1024 DoubleRow fp8 matmuls (N=512) x 216 ns = 221 us
plus 3.5 us of fp16 gating matmuls; everything else is scheduled to hide
under that:
  - all inputs are pre-arranged on the HOST into the exact SBUF layout
    [128 partitions, subtile, col], so every DMA is a wide contiguous
    copy (max line size, cheap descriptors),
  - two HWDGE queues run concurrently: expert weights stream on the Sync
    queue, x / gating inputs on the Scalar queue,
  - ~16 junk warm-up matmuls run during the initial DMA wait so the PE's
    HAM clock-gate is already at 8/8 when the real matmuls start,
  - the per-expert combine (acc += gate * y) reads PSUM directly on DVE
    (skipping the reference's bf16 round-trip costs ~1e-3 rel err, well
    inside the 2e-2 budget) - no y tiles, fewer semaphores, shorter
    compiler epilogue,
  - expert 0 runs before gating is known: its PSUM is copied raw into
    acc (ACT engine), then scaled in place by gate[:,0] once the softmax
    (computed at the phase boundary inside expert 0) is done,
  - the last expert runs hc-major within each mc so each PSUM bank
    finishes 8 matmuls before the next, letting the combine + output DMA
    drain while the PE is still working; out pieces alternate between
    the two HWDGE queues.
"""

import numpy as np
import ml_dtypes

import concourse.bacc as bacc
import concourse.bass as bass
import concourse.mybir as mybir
import concourse.tile as tile
from concourse import masks
from concourse.bass_utils import run_bass_kernel_spmd

M, D, H, E = 4096, 2048, 2048, 8
NCORES = 8
MS = M // NCORES          # tokens per core (512)
MC = MS // 128            # m-chunks of 128 partitions (4)
DS = D // 128             # d-subtiles of 128 (16)
KP = DS // 2              # DoubleRow d-pairs of 256 (8)
NH = 512                  # h columns per matmul (one PSUM bank of f32)
HC = H // NH              # h-chunks (4)
NWARM = 20                # junk warm-up matmuls (N=256)

_NC = None


def _build_program() -> bass.Bass:
    dt = mybir.dt
    nc = bacc.Bacc(None, target_bir_lowering=False)

    # Host pre-arranges everything into [128, subtile, col] SBUF order.
    xq = nc.dram_tensor("xq", [128, DS * MS], dt.float8e4, kind="ExternalInput")
    xf = nc.dram_tensor("xf", [128, DS * MS], dt.float16, kind="ExternalInput")
    wq = nc.dram_tensor("wq", [E * 128, DS * H], dt.float8e4, kind="ExternalInput")
    gwt = nc.dram_tensor("gwt", [128, DS * E], dt.float16, kind="ExternalInput")
    gb = nc.dram_tensor("gb", [E, 1], dt.float32, kind="ExternalInput")
    out = nc.dram_tensor("out", [MS, H], dt.float32, kind="ExternalOutput")

    with tile.TileContext(nc) as tc, \
            tc.tile_pool(name="const", bufs=1) as constp, \
            tc.tile_pool(name="wpool", bufs=3) as wpool, \
            tc.tile_pool(name="small", bufs=8) as small, \
            tc.tile_pool(name="ps", bufs=8, space="PSUM") as psp:

        # Persistent SBUF tensors. Contraction index d = s*128 + p.
        xq_sb = constp.tile([128, DS, MS], dt.float8e4, tag="xq")
        xf_sb = constp.tile([128, DS, MS], dt.float16, tag="xf")
        gwt_sb = constp.tile([128, DS, E], dt.float16, tag="gwt")
        gb_sb = constp.tile([E, 1], dt.float32, tag="gb")
        id8_sb = constp.tile([E, E], dt.float32, tag="id8")
        gate_sb = constp.tile([128, MC * E], dt.float32, tag="gate")
        lg_sb = constp.tile([E, MS], dt.float32, tag="lg")
        acc_sb = constp.tile([128, MC * H], dt.float32, tag="acc")
        junk_sb = constp.tile([128, 384], dt.float8e4, tag="junk")

        # Warm-up matmuls: only dependence is the junk memset (DVE,
        # right after its preamble), so they start immediately and carry
        # the PE through the HAM activity window while the first real
        # operands stream in.
        nc.vector.memset(junk_sb[:], 0.0)
        ps_w = psp.tile([128, NH], dt.float32, tag="ps", name="ps_warm")
        for i in range(NWARM):
            nc.tensor.matmul(
                ps_w[:, 0:256],
                lhsT=junk_sb[:, 0:128],
                rhs=junk_sb[:, 128:384],
                start=True,
                stop=True,
            )

        # Startup DMA: aggregate HBM delivery is ~290 GB/s and phase 1
        # needs xq + all of w0 (4.5 MB), so both HWDGE queues carry it:
        # Sync (fast start) gets xq plus the even k-pair slices of w0,
        # Scalar gets the odd ones. The gating inputs are chained behind
        # Scalar's last w0 slice so they don't steal phase-1 bandwidth.
        with tc.high_priority():
            nc.sync.dma_start(
                xq_sb[:, 0:8, :],
                xq[:, 0:8 * MS].rearrange("p (s m) -> p s m", m=MS))
            nc.sync.dma_start(
                xq_sb[:, 8:DS, :],
                xq[:, 8 * MS:].rearrange("p (s m) -> p s m", m=MS))
        nc.gpsimd.dma_start(gb_sb[:], gb[:, :])
        masks.make_identity(nc, id8_sb[:])

        def emit_gating():
            ps_gt = psp.tile([E, MS], dt.float32, tag="ps", name="ps_gt")
            for s in range(DS):
                nc.tensor.matmul(
                    ps_gt[:],
                    lhsT=gwt_sb[:, s:s + 1, :],
                    rhs=xf_sb[:, s:s + 1, :],
                    start=(s == 0),
                    stop=(s == DS - 1),
                )
            nc.vector.tensor_scalar_add(lg_sb[:], ps_gt[:], gb_sb[:])

        def emit_softmax():
            for mc in range(MC):
                pst = psp.tile([128, E], dt.float32, tag="ps", name=f"ps_t{mc}")
                nc.tensor.transpose(
                    pst[:], lg_sb[:, mc * 128:(mc + 1) * 128], id8_sb[:]
                )
                mx = small.tile([128, 1], dt.float32, tag="sm1")
                nc.vector.tensor_reduce(
                    mx[:], pst[:], mybir.AxisListType.X, mybir.AluOpType.max
                )
                nmx = small.tile([128, 1], dt.float32, tag="sm1")
                nc.vector.tensor_scalar_mul(nmx[:], mx[:], -1.0)
                ex = small.tile([128, E], dt.float32, tag="sm")
                ssum = small.tile([128, 1], dt.float32, tag="sm1")
                nc.scalar.activation(
                    ex[:], pst[:], mybir.ActivationFunctionType.Exp,
                    bias=nmx[:], scale=1.0, accum_out=ssum[:],
                )
                rcp = small.tile([128, 1], dt.float32, tag="sm1")
                nc.vector.reciprocal(rcp[:], ssum[:])
                nc.vector.tensor_scalar_mul(
                    gate_sb[:, mc * E:(mc + 1) * E], ex[:], rcp[:]
                )

        # ---- Main GEMM + weighted combine ----
        for e in range(E):
            w_sb = wpool.tile([128, DS, H], dt.float8e4, tag="w")
            if e == 0:
                # w0 streams as 8 k-pair slices alternating between the
                # two HWDGE queues (even -> Sync, odd -> Scalar) so both
                # queues deliver in phase-1 consumption order.
                w0_scalar = []
                for kp in range(KP):
                    eng = nc.sync if kp % 2 == 0 else nc.scalar
                    dj = eng.dma_start(
                        w_sb[:, 2 * kp:2 * kp + 2, :],
                        wq[0:128, kp * 2 * H:(kp + 1) * 2 * H].rearrange(
                            "p (s h) -> p s h", h=H),
                    )
                    if kp % 2 == 1:
                        w0_scalar.append(dj)
                # Gating inputs behind Scalar's last w0 slice.
                from concourse.tile import add_dep_helper
                for j in range(2):
                    dxf = nc.scalar.dma_start(
                        xf_sb[:, j * 8:(j + 1) * 8, :],
                        xf[:, j * 8 * MS:(j + 1) * 8 * MS].rearrange(
                            "p (s m) -> p s m", m=MS),
                    )
                    if j == 0:
                        add_dep_helper(dxf.ins, w0_scalar[-1].ins,
                                       reason="xf after w0")
                nc.scalar.dma_start(
                    gwt_sb[:], gwt[:, :].rearrange("p (s e) -> p s e", e=E))
            else:
                # Later experts load whole (fewer sem waits, which
                # otherwise split into extra LDWEIGHTS slots).
                nc.sync.dma_start(
                    w_sb[:],
                    wq[e * 128:(e + 1) * 128, :].rearrange(
                        "p (s h) -> p s h", h=H),
                )
            if e == 0:
                # Expert 0 runs as two mc-pair phases (k-major, 8 PSUM
                # banks each) consuming w0 pieces at arrival rate. Raw
                # PSUM is copied to acc on ACT; the gate scale is applied
                # in place after softmax. Gating matmuls sit between the
                # phases where the DMA pressure peaks.
                # Phase (0,1) runs k-major across the mc pair so it can
                # consume w0 pieces at arrival rate; mc 2 and 3 run
                # sequentially afterwards (no DMA dependence) so their
                # first PSUM banks free up before the softmax transposes
                # need the slots.
                for phase_mcs in ((0, 1), (2,), (3,)):
                    pss = {
                        mc: [
                            psp.tile([128, NH], dt.float32, tag="ps",
                                     name=f"ps0_{mc}_{i}")
                            for i in range(HC)
                        ]
                        for mc in phase_mcs
                    }
                    for k in range(KP):
                        for mc in phase_mcs:
                            lhsT = xq_sb[:, 2 * k:2 * k + 2,
                                         mc * 128:(mc + 1) * 128]
                            for hc in range(HC):
                                nc.tensor.matmul(
                                    pss[mc][hc][:],
                                    lhsT=lhsT,
                                    rhs=w_sb[:, 2 * k:2 * k + 2,
                                             hc * NH:(hc + 1) * NH],
                                    start=(k == 0),
                                    stop=(k == KP - 1),
                                    perf_mode=mybir.MatmulPerfMode.DoubleRow,
                                )
                    for mc in phase_mcs:
                        for hc in range(HC):
                            nc.scalar.copy(
                                acc_sb[:, mc * H + hc * NH:
                                       mc * H + (hc + 1) * NH],
                                pss[mc][hc][:],
                            )
                # Gating at the e0/e1 boundary: mc 2's PSUM banks freed
                # early (sequential phases) so the softmax transposes get
                # slots without stalling the PE.
                emit_gating()
                emit_softmax()
                for mc in range(MC):
                    g0_ap = gate_sb[:, mc * E:mc * E + 1]
                    for hc in range(HC):
                        a_ap = acc_sb[:, mc * H + hc * NH:
                                      mc * H + (hc + 1) * NH]
                        nc.scalar.mul(a_ap, a_ap, g0_ap)
                continue

            last = e == E - 1
            for mc in range(MC):
                msl = slice(mc * 128, (mc + 1) * 128)
                pss = [
                    psp.tile([128, NH], dt.float32, tag="ps",
                             name=f"ps_{e}_{mc}_{i}")
                    for i in range(HC)
                ]
                g_ap = gate_sb[:, mc * E + e:mc * E + e + 1]
                if last:
                    # hc-major: each bank completes its 8-matmul k-loop
                    # before the next starts, so combine + out DMA drain
                    # behind the PE instead of after it. The very last
                    # chunk is split in two so the post-matmul tail is a
                    # 256-col combine plus a 128 KB DMA.
                    for hc in range(HC):
                        for k in range(KP):
                            nc.tensor.matmul(
                                pss[hc][:],
                                lhsT=xq_sb[:, 2 * k:2 * k + 2, msl],
                                rhs=w_sb[:, 2 * k:2 * k + 2,
                                         hc * NH:(hc + 1) * NH],
                                start=(k == 0),
                                stop=(k == KP - 1),
                                perf_mode=mybir.MatmulPerfMode.DoubleRow,
                            )
                        final = mc == MC - 1 and hc == HC - 1
                        for q in range(2) if final else range(1):
                            csl = slice(hc * NH + (NH // 2) * q,
                                        hc * NH + (NH // 2) * (q + 1)
                                        ) if final else slice(
                                            hc * NH, (hc + 1) * NH)
                            a_ap = acc_sb[:, mc * H + csl.start:
                                          mc * H + csl.stop]
                            p_ap = pss[hc][:, csl.start - hc * NH:
                                           csl.stop - hc * NH]
                            nc.vector.scalar_tensor_tensor(
                                a_ap, p_ap, g_ap, a_ap,
                                op0=mybir.AluOpType.mult,
                                op1=mybir.AluOpType.add,
                            )
                            eng = nc.scalar if (mc * HC + hc) % 2 == 0 \
                                else nc.sync
                            eng.dma_start(
                                out[mc * 128:(mc + 1) * 128, csl], a_ap)
                else:
                    for k in range(KP):
                        lhsT = xq_sb[:, 2 * k:2 * k + 2, msl]
                        for hc in range(HC):
                            nc.tensor.matmul(
                                pss[hc][:],
                                lhsT=lhsT,
                                rhs=w_sb[:, 2 * k:2 * k + 2,
                                         hc * NH:(hc + 1) * NH],
                                start=(k == 0),
                                stop=(k == KP - 1),
                                perf_mode=mybir.MatmulPerfMode.DoubleRow,
                            )
                    for hc in range(HC):
                        a_ap = acc_sb[:, mc * H + hc * NH:
                                      mc * H + (hc + 1) * NH]
                        nc.vector.scalar_tensor_tensor(
                            a_ap, pss[hc][:], g_ap, a_ap,
                            op0=mybir.AluOpType.mult,
                            op1=mybir.AluOpType.add,
                        )

    nc.compile()
    return nc


def _get_nc() -> bass.Bass:
    global _NC
    if _NC is None:
        _NC = _build_program()
    return _NC


def _sbuf_order(a, cols):
    """[D, cols] -> [128, DS_a * cols] contiguous in (p, s, col) order."""
    d = a.shape[0]
    return np.ascontiguousarray(
        a.reshape(d // 128, 128, cols).transpose(1, 0, 2)
    ).reshape(128, (d // 128) * cols)


def _prep_in_maps(x, gate_w, gate_b, expert_w):
    f8fn = ml_dtypes.float8_e4m3fn
    f8trn = ml_dtypes.float8_e4m3  # same bits as e4m3fn for |v| <= 240

    x = np.asarray(x, dtype=np.float32)
    gate_w = np.asarray(gate_w, dtype=np.float32)
    gate_b = np.asarray(gate_b, dtype=np.float32)
    expert_w = np.asarray(expert_w, dtype=np.float32)

    xT = np.ascontiguousarray(x.T)                       # [D, M] f32
    xT_f16 = xT.astype(np.float16)                       # gating copy
    xqT = xT.astype(f8fn).view(f8trn)                    # [D, M] fp8
    # expert_w [E, H, D] -> per-expert w^T [D, H], quantized, then into
    # SBUF order [E*128, DS*H].
    wqT = np.ascontiguousarray(
        expert_w.transpose(0, 2, 1)
    ).astype(f8fn).view(f8trn)                           # [E, D, H]
    wq_l = np.ascontiguousarray(
        wqT.reshape(E, DS, 128, H).transpose(0, 2, 1, 3)
    ).reshape(E * 128, DS * H)
    gwt_l = _sbuf_order(
        np.ascontiguousarray(gate_w.T).astype(np.float16), E)
    gbb = np.ascontiguousarray(gate_b.reshape(E, 1))

    in_maps = []
    for c in range(NCORES):
        csl = slice(c * MS, (c + 1) * MS)
        in_maps.append({
            "xq": _sbuf_order(np.ascontiguousarray(xqT[:, csl]), MS),
            "xf": _sbuf_order(np.ascontiguousarray(xT_f16[:, csl]), MS),
            "wq": wq_l,
            "gwt": gwt_l,
            "gb": gbb,
        })
    return in_maps


def kernel(x, gate_w, gate_b, expert_w, _trace=False, _trace_kwargs=None):
    nc = _get_nc()
    in_maps = _prep_in_maps(x, gate_w, gate_b, expert_w)
    kw = {}
    if _trace:
        kw["trace"] = True
        kw.update(_trace_kwargs or {})
    res = run_bass_kernel_spmd(nc, in_maps, core_ids=list(range(NCORES)), **kw)
    outp = np.concatenate(
        [np.asarray(res.results[c]["out"]) for c in range(NCORES)], axis=0
    )
    if _trace:
        return outp, res
    return outp


# revision 18
# speedup vs baseline: 1.0070x; 1.0070x over previous
"""MoE-with-DeepGEMM kernel for 8 Trainium2 NeuronCores.

Problem: M=4096 tokens, D=2048 in-dim, H=2048 out-dim, E=8 experts.
    gate = softmax(x @ gate_w.T + gate_b)            # [M, E], fp32
    y    = (q8(x) @ q8(expert_w[e]).T) -> bf16       # [E, M, H]
    out  = sum_e gate[:, e, None] * y[e].astype(f32) # [M, H]

Strategy: data-parallel over tokens (M). Each of the 8 cores gets
M/8 = 512 tokens, all 8 experts' weights, and computes its output slice
independently - no collectives; the host concatenates the slices.

The PE floor is 1024 DoubleRow fp8 matmuls (N=512) x 216 ns = 221 us
plus 3.5 us of fp16 gating matmuls; everything else is scheduled to hide
under that:
  - all inputs are pre-arranged on the HOST into the exact SBUF layout
    [128 partitions, subtile, col], so every DMA is a wide contiguous
    copy (max line size, cheap descriptors),
  - two HWDGE queues run concurrently: expert weights stream on the Sync
    queue, x / gating inputs on the Scalar queue,
  - ~16 junk warm-up matmuls run during the initial DMA wait so the PE's
    HAM clock-gate is already at 8/8 when the real matmuls start,
  - the per-expert combine (acc += gate * y) reads PSUM directly on DVE
    (skipping the reference's bf16 round-trip costs ~1e-3 rel err, well
    inside the 2e-2 budget) - no y tiles, fewer semaphores, shorter
    compiler epilogue,
  - expert 0 runs before gating is known: its PSUM is copied raw into
    acc (ACT engine), then scaled in place by gate[:,0] once the softmax
    (computed at the phase boundary inside expert 0) is done,
  - the last expert runs hc-major within each mc so each PSUM bank
    finishes 8 matmuls before the next, letting the combine + output DMA
    drain while the PE is still working; out pieces alternate between
    the two HWDGE queues.
"""

import numpy as np
import ml_dtypes

import concourse.bacc as bacc
import concourse.bass as bass
import concourse.mybir as mybir
import concourse.tile as tile
from concourse import masks
from concourse.bass_utils import run_bass_kernel_spmd

M, D, H, E = 4096, 2048, 2048, 8
NCORES = 8
MS = M // NCORES          # tokens per core (512)
MC = MS // 128            # m-chunks of 128 partitions (4)
DS = D // 128             # d-subtiles of 128 (16)
KP = DS // 2              # DoubleRow d-pairs of 256 (8)
NH = 512                  # h columns per matmul (one PSUM bank of f32)
HC = H // NH              # h-chunks (4)
NWARM = 20                # junk warm-up matmuls (N=256)

_NC = None


def _build_program() -> bass.Bass:
    dt = mybir.dt
    nc = bacc.Bacc(None, target_bir_lowering=False)

    # Host pre-arranges everything into [128, subtile, col] SBUF order.
    xq = nc.dram_tensor("xq", [128, DS * MS], dt.float8e4, kind="ExternalInput")
    xf = nc.dram_tensor("xf", [128, DS * MS], dt.float16, kind="ExternalInput")
    wq = nc.dram_tensor("wq", [E * 128, DS * H], dt.float8e4, kind="ExternalInput")
    gwt = nc.dram_tensor("gwt", [128, DS * E], dt.float16, kind="ExternalInput")
    gb = nc.dram_tensor("gb", [E, 1], dt.float32, kind="ExternalInput")
    out = nc.dram_tensor("out", [MS, H], dt.float32, kind="ExternalOutput")

    with tile.TileContext(nc) as tc, \
            tc.tile_pool(name="const", bufs=1) as constp, \
            tc.tile_pool(name="wpool", bufs=3) as wpool, \
            tc.tile_pool(name="small", bufs=8) as small, \
            tc.tile_pool(name="ps", bufs=8, space="PSUM") as psp:

        # Persistent SBUF tensors. Contraction index d = s*128 + p.
        xq_sb = constp.tile([128, DS, MS], dt.float8e4, tag="xq")
        xf_sb = constp.tile([128, DS, MS], dt.float16, tag="xf")
        gwt_sb = constp.tile([128, DS, E], dt.float16, tag="gwt")
        gb_sb = constp.tile([E, 1], dt.float32, tag="gb")
        id8_sb = constp.tile([E, E], dt.float32, tag="id8")
        gate_sb = constp.tile([128, MC * E], dt.float32, tag="gate")
        lg_sb = constp.tile([E, MS], dt.float32, tag="lg")
        acc_sb = constp.tile([128, MC * H], dt.float32, tag="acc")
        junk_sb = constp.tile([128, 384], dt.float8e4, tag="junk")

        # Warm-up matmuls: only dependence is the junk memset (DVE,
        # right after its preamble), so they start immediately and carry
        # the PE through the HAM activity window while the first real
        # operands stream in.
        nc.vector.memset(junk_sb[:], 0.0)
        ps_w = psp.tile([128, NH], dt.float32, tag="ps", name="ps_warm")
        for i in range(NWARM):
            nc.tensor.matmul(
                ps_w[:, 0:256],
                lhsT=junk_sb[:, 0:128],
                rhs=junk_sb[:, 128:384],
                start=True,
                stop=True,
            )

        # Startup DMA: aggregate HBM delivery is ~290 GB/s and phase 1
        # needs xq + all of w0 (4.5 MB), so both HWDGE queues carry it:
        # Sync (fast start) gets xq plus the even k-pair slices of w0,
        # Scalar gets the odd ones. The gating inputs are chained behind
        # Scalar's last w0 slice so they don't steal phase-1 bandwidth.
        nc.gpsimd.dma_start(gb_sb[:], gb[:, :])
        masks.make_identity(nc, id8_sb[:])

        def dma_xq(lo, hi):
            return nc.sync.dma_start(
                xq_sb[:, lo:hi, :],
                xq[:, lo * MS:hi * MS].rearrange("p (s m) -> p s m", m=MS))

        def emit_gating():
            ps_gt = psp.tile([E, MS], dt.float32, tag="ps", name="ps_gt")
            for s in range(DS):
                nc.tensor.matmul(
                    ps_gt[:],
                    lhsT=gwt_sb[:, s:s + 1, :],
                    rhs=xf_sb[:, s:s + 1, :],
                    start=(s == 0),
                    stop=(s == DS - 1),
                )
            nc.vector.tensor_scalar_add(lg_sb[:], ps_gt[:], gb_sb[:])

        def emit_softmax():
            for mc in range(MC):
                pst = psp.tile([128, E], dt.float32, tag="ps", name=f"ps_t{mc}")
                nc.tensor.transpose(
                    pst[:], lg_sb[:, mc * 128:(mc + 1) * 128], id8_sb[:]
                )
                mx = small.tile([128, 1], dt.float32, tag="sm1")
                nc.vector.tensor_reduce(
                    mx[:], pst[:], mybir.AxisListType.X, mybir.AluOpType.max
                )
                nmx = small.tile([128, 1], dt.float32, tag="sm1")
                nc.vector.tensor_scalar_mul(nmx[:], mx[:], -1.0)
                ex = small.tile([128, E], dt.float32, tag="sm")
                ssum = small.tile([128, 1], dt.float32, tag="sm1")
                nc.scalar.activation(
                    ex[:], pst[:], mybir.ActivationFunctionType.Exp,
                    bias=nmx[:], scale=1.0, accum_out=ssum[:],
                )
                rcp = small.tile([128, 1], dt.float32, tag="sm1")
                nc.vector.reciprocal(rcp[:], ssum[:])
                nc.vector.tensor_scalar_mul(
                    gate_sb[:, mc * E:(mc + 1) * E], ex[:], rcp[:]
                )

        # ---- Main GEMM + weighted combine ----
        for e in range(E):
            w_sb = wpool.tile([128, DS, H], dt.float8e4, tag="w")
            if e == 0:
                # Startup feed: everything phase 1 needs goes down the
                # Sync queue (fast start, strict FIFO) interleaved in
                # exact consumption order - xq s-slices just ahead of the
                # w0 k-pair slices that are multiplied against them. The
                # gating inputs ride the Scalar queue, released once the
                # back half of w0 is underway so they don't steal
                # startup bandwidth.
                from concourse.tile import add_dep_helper
                w0p = []
                dma_xq(0, 4)
                for kp in range(KP):
                    if kp in (2, 4, 6):
                        dma_xq(2 * kp, 2 * kp + 4)
                    w0p.append(nc.sync.dma_start(
                        w_sb[:, 2 * kp:2 * kp + 2, :],
                        wq[0:128, kp * 2 * H:(kp + 1) * 2 * H].rearrange(
                            "p (s h) -> p s h", h=H),
                    ))
                for j in range(2):
                    dxf = nc.scalar.dma_start(
                        xf_sb[:, j * 8:(j + 1) * 8, :],
                        xf[:, j * 8 * MS:(j + 1) * 8 * MS].rearrange(
                            "p (s m) -> p s m", m=MS),
                    )
                    if j == 0:
                        add_dep_helper(dxf.ins, w0p[4].ins,
                                       reason="xf after w0 front half")
                nc.scalar.dma_start(
                    gwt_sb[:], gwt[:, :].rearrange("p (s e) -> p s e", e=E))
            else:
                # Later experts load whole (fewer sem waits, which
                # otherwise split into extra LDWEIGHTS slots).
                nc.sync.dma_start(
                    w_sb[:],
                    wq[e * 128:(e + 1) * 128, :].rearrange(
                        "p (s h) -> p s h", h=H),
                )
            if e == 0:
                # Expert 0 runs as two mc-pair phases (k-major, 8 PSUM
                # banks each) consuming w0 pieces at arrival rate. Raw
                # PSUM is copied to acc on ACT; the gate scale is applied
                # in place after softmax. Gating matmuls sit between the
                # phases where the DMA pressure peaks.
                # Phase (0,1) runs k-major across the mc pair so it can
                # consume w0 pieces at arrival rate; mc 2 and 3 run
                # sequentially afterwards (no DMA dependence) so their
                # first PSUM banks free up before the softmax transposes
                # need the slots.
                for phase_mcs in ((0, 1), (2,), (3,)):
                    pss = {
                        mc: [
                            psp.tile([128, NH], dt.float32, tag="ps",
                                     name=f"ps0_{mc}_{i}")
                            for i in range(HC)
                        ]
                        for mc in phase_mcs
                    }
                    for k in range(KP):
                        for mc in phase_mcs:
                            lhsT = xq_sb[:, 2 * k:2 * k + 2,
                                         mc * 128:(mc + 1) * 128]
                            for hc in range(HC):
                                nc.tensor.matmul(
                                    pss[mc][hc][:],
                                    lhsT=lhsT,
                                    rhs=w_sb[:, 2 * k:2 * k + 2,
                                             hc * NH:(hc + 1) * NH],
                                    start=(k == 0),
                                    stop=(k == KP - 1),
                                    perf_mode=mybir.MatmulPerfMode.DoubleRow,
                                )
                    for mc in phase_mcs:
                        for hc in range(HC):
                            nc.scalar.copy(
                                acc_sb[:, mc * H + hc * NH:
                                       mc * H + (hc + 1) * NH],
                                pss[mc][hc][:],
                            )
                # Gating at the e0/e1 boundary: mc 2's PSUM banks freed
                # early (sequential phases) so the softmax transposes get
                # slots without stalling the PE.
                emit_gating()
                emit_softmax()
                for mc in range(MC):
                    g0_ap = gate_sb[:, mc * E:mc * E + 1]
                    for hc in range(HC):
                        a_ap = acc_sb[:, mc * H + hc * NH:
                                      mc * H + (hc + 1) * NH]
                        nc.scalar.mul(a_ap, a_ap, g0_ap)
                continue

            last = e == E - 1
            for mc in range(MC):
                msl = slice(mc * 128, (mc + 1) * 128)
                pss = [
                    psp.tile([128, NH], dt.float32, tag="ps",
                             name=f"ps_{e}_{mc}_{i}")
                    for i in range(HC)
                ]
                g_ap = gate_sb[:, mc * E + e:mc * E + e + 1]
                if last:
                    # hc-major: each bank completes its 8-matmul k-loop
                    # before the next starts, so combine + out DMA drain
                    # behind the PE instead of after it. The very last
                    # chunk is split in two so the post-matmul tail is a
                    # 256-col combine plus a 128 KB DMA.
                    for hc in range(HC):
                        for k in range(KP):
                            nc.tensor.matmul(
                                pss[hc][:],
                                lhsT=xq_sb[:, 2 * k:2 * k + 2, msl],
                                rhs=w_sb[:, 2 * k:2 * k + 2,
                                         hc * NH:(hc + 1) * NH],
                                start=(k == 0),
                                stop=(k == KP - 1),
                                perf_mode=mybir.MatmulPerfMode.DoubleRow,
                            )
                        final = mc == MC - 1 and hc == HC - 1
                        for q in range(2) if final else range(1):
                            csl = slice(hc * NH + (NH // 2) * q,
                                        hc * NH + (NH // 2) * (q + 1)
                                        ) if final else slice(
                                            hc * NH, (hc + 1) * NH)
                            a_ap = acc_sb[:, mc * H + csl.start:
                                          mc * H + csl.stop]
                            p_ap = pss[hc][:, csl.start - hc * NH:
                                           csl.stop - hc * NH]
                            nc.vector.scalar_tensor_tensor(
                                a_ap, p_ap, g_ap, a_ap,
                                op0=mybir.AluOpType.mult,
                                op1=mybir.AluOpType.add,
                            )
                            eng = nc.scalar if (mc * HC + hc) % 2 == 0 \
                                else nc.sync
                            eng.dma_start(
                                out[mc * 128:(mc + 1) * 128, csl], a_ap)
                else:
                    for k in range(KP):
                        lhsT = xq_sb[:, 2 * k:2 * k + 2, msl]
                        for hc in range(HC):
                            nc.tensor.matmul(
                                pss[hc][:],
                                lhsT=lhsT,
                                rhs=w_sb[:, 2 * k:2 * k + 2,
                                         hc * NH:(hc + 1) * NH],
                                start=(k == 0),
                                stop=(k == KP - 1),
                                perf_mode=mybir.MatmulPerfMode.DoubleRow,
                            )
                    for hc in range(HC):
                        a_ap = acc_sb[:, mc * H + hc * NH:
                                      mc * H + (hc + 1) * NH]
                        nc.vector.scalar_tensor_tensor(
                            a_ap, pss[hc][:], g_ap, a_ap,
                            op0=mybir.AluOpType.mult,
                            op1=mybir.AluOpType.add,
                        )

    nc.compile()
    return nc


def _get_nc() -> bass.Bass:
    global _NC
    if _NC is None:
        _NC = _build_program()
    return _NC


def _sbuf_order(a, cols):
    """[D, cols] -> [128, DS_a * cols] contiguous in (p, s, col) order."""
    d = a.shape[0]
    return np.ascontiguousarray(
        a.reshape(d // 128, 128, cols).transpose(1, 0, 2)
    ).reshape(128, (d // 128) * cols)


def _prep_in_maps(x, gate_w, gate_b, expert_w):
    f8fn = ml_dtypes.float8_e4m3fn
    f8trn = ml_dtypes.float8_e4m3  # same bits as e4m3fn for |v| <= 240

    x = np.asarray(x, dtype=np.float32)
    gate_w = np.asarray(gate_w, dtype=np.float32)
    gate_b = np.asarray(gate_b, dtype=np.float32)
    expert_w = np.asarray(expert_w, dtype=np.float32)

    xT = np.ascontiguousarray(x.T)                       # [D, M] f32
    xT_f16 = xT.astype(np.float16)                       # gating copy
    xqT = xT.astype(f8fn).view(f8trn)                    # [D, M] fp8
    # expert_w [E, H, D] -> per-expert w^T [D, H], quantized, then into
    # SBUF order [E*128, DS*H].
    wqT = np.ascontiguousarray(
        expert_w.transpose(0, 2, 1)
    ).astype(f8fn).view(f8trn)                           # [E, D, H]
    wq_l = np.ascontiguousarray(
        wqT.reshape(E, DS, 128, H).transpose(0, 2, 1, 3)
    ).reshape(E * 128, DS * H)
    gwt_l = _sbuf_order(
        np.ascontiguousarray(gate_w.T).astype(np.float16), E)
    gbb = np.ascontiguousarray(gate_b.reshape(E, 1))

    in_maps = []
    for c in range(NCORES):
        csl = slice(c * MS, (c + 1) * MS)
        in_maps.append({
            "xq": _sbuf_order(np.ascontiguousarray(xqT[:, csl]), MS),
            "xf": _sbuf_order(np.ascontiguousarray(xT_f16[:, csl]), MS),
            "wq": wq_l,
            "gwt": gwt_l,
            "gb": gbb,
        })
    return in_maps


def kernel(x, gate_w, gate_b, expert_w, _trace=False, _trace_kwargs=None):
    nc = _get_nc()
    in_maps = _prep_in_maps(x, gate_w, gate_b, expert_w)
    kw = {}
    if _trace:
        kw["trace"] = True
        kw.update(_trace_kwargs or {})
    res = run_bass_kernel_spmd(nc, in_maps, core_ids=list(range(NCORES)), **kw)
    outp = np.concatenate(
        [np.asarray(res.results[c]["out"]) for c in range(NCORES)], axis=0
    )
    if _trace:
        return outp, res
    return outp


# revision 20
# speedup vs baseline: 1.0342x; 1.0270x over previous
"""MoE-with-DeepGEMM kernel for 8 Trainium2 NeuronCores.

Problem: M=4096 tokens, D=2048 in-dim, H=2048 out-dim, E=8 experts.
    gate = softmax(x @ gate_w.T + gate_b)            # [M, E], fp32
    y    = (q8(x) @ q8(expert_w[e]).T) -> bf16       # [E, M, H]
    out  = sum_e gate[:, e, None] * y[e].astype(f32) # [M, H]

Strategy: data-parallel over tokens (M). Each of the 8 cores gets
M/8 = 512 tokens, all 8 experts' weights, and computes its output slice
independently - no collectives; the host concatenates the slices.

The PE floor is 1024 DoubleRow fp8 matmuls (N=512) x 216 ns = 221 us
plus 3.5 us of fp16 gating matmuls; everything else is scheduled to hide
under that:
  - all inputs are pre-arranged on the HOST into the exact SBUF layout
    [128 partitions, subtile, col], so every DMA is a wide contiguous
    copy (max line size, cheap descriptors),
  - two HWDGE queues run concurrently: expert weights stream on the Sync
    queue, x / gating inputs on the Scalar queue,
  - ~16 junk warm-up matmuls run during the initial DMA wait so the PE's
    HAM clock-gate is already at 8/8 when the real matmuls start,
  - the per-expert combine (acc += gate * y) reads PSUM directly on DVE
    (skipping the reference's bf16 round-trip costs ~1e-3 rel err, well
    inside the 2e-2 budget) - no y tiles, fewer semaphores, shorter
    compiler epilogue,
  - expert 0 runs before gating is known: its PSUM is copied raw into
    acc (ACT engine), then scaled in place by gate[:,0] once the softmax
    (computed at the phase boundary inside expert 0) is done,
  - the last expert runs hc-major within each mc so each PSUM bank
    finishes 8 matmuls before the next, letting the combine + output DMA
    drain while the PE is still working; out pieces alternate between
    the two HWDGE queues.
"""

import numpy as np
import ml_dtypes

import concourse.bacc as bacc
import concourse.bass as bass
import concourse.mybir as mybir
import concourse.tile as tile
from concourse import masks
from concourse.bass_utils import run_bass_kernel_spmd

M, D, H, E = 4096, 2048, 2048, 8
NCORES = 8
MS = M // NCORES          # tokens per core (512)
MC = MS // 128            # m-chunks of 128 partitions (4)
DS = D // 128             # d-subtiles of 128 (16)
KP = DS // 2              # DoubleRow d-pairs of 256 (8)
NH = 512                  # h columns per matmul (one PSUM bank of f32)
HC = H // NH              # h-chunks (4)
NWARM = 24                # junk warm-up matmuls (N=256)

_NC = None


def _build_program() -> bass.Bass:
    dt = mybir.dt
    nc = bacc.Bacc(None, target_bir_lowering=False)

    # Host pre-arranges everything into [128, subtile, col] SBUF order.
    xq = nc.dram_tensor("xq", [128, DS * MS], dt.float8e4, kind="ExternalInput")
    xf = nc.dram_tensor("xf", [128, DS * MS], dt.float16, kind="ExternalInput")
    wq = nc.dram_tensor("wq", [E * 128, DS * H], dt.float8e4, kind="ExternalInput")
    gwt = nc.dram_tensor("gwt", [128, DS * E], dt.float16, kind="ExternalInput")
    gb = nc.dram_tensor("gb", [E, 1], dt.float32, kind="ExternalInput")
    out = nc.dram_tensor("out", [MS, H], dt.float32, kind="ExternalOutput")

    with tile.TileContext(nc) as tc, \
            tc.tile_pool(name="const", bufs=1) as constp, \
            tc.tile_pool(name="wpool", bufs=3) as wpool, \
            tc.tile_pool(name="small", bufs=8) as small, \
            tc.tile_pool(name="ps", bufs=8, space="PSUM") as psp:

        # Persistent SBUF tensors. Contraction index d = s*128 + p.
        xq_sb = constp.tile([128, DS, MS], dt.float8e4, tag="xq")
        xf_sb = constp.tile([128, DS, MS], dt.float16, tag="xf")
        gwt_sb = constp.tile([128, DS, E], dt.float16, tag="gwt")
        gb_sb = constp.tile([E, 1], dt.float32, tag="gb")
        id8_sb = constp.tile([E, E], dt.float32, tag="id8")
        gate_sb = constp.tile([128, MC * E], dt.float32, tag="gate")
        lg_sb = constp.tile([E, MS], dt.float32, tag="lg")
        acc_sb = constp.tile([128, MC * H], dt.float32, tag="acc")
        junk_sb = constp.tile([128, 384], dt.float8e4, tag="junk")

        # Warm-up matmuls: only dependence is the junk memset (DVE,
        # right after its preamble), so they start immediately and carry
        # the PE through the HAM activity window while the first real
        # operands stream in.
        nc.vector.memset(junk_sb[:], 0.0)
        ps_w = psp.tile([128, NH], dt.float32, tag="ps", name="ps_warm")
        for i in range(NWARM):
            nc.tensor.matmul(
                ps_w[:, 0:256],
                lhsT=junk_sb[:, 0:128],
                rhs=junk_sb[:, 128:384],
                start=True,
                stop=True,
            )

        # Startup DMA: aggregate HBM delivery is ~290 GB/s and phase 1
        # needs xq + all of w0 (4.5 MB), so both HWDGE queues carry it:
        # Sync (fast start) gets xq plus the even k-pair slices of w0,
        # Scalar gets the odd ones. The gating inputs are chained behind
        # Scalar's last w0 slice so they don't steal phase-1 bandwidth.
        nc.gpsimd.dma_start(gb_sb[:], gb[:, :])
        masks.make_identity(nc, id8_sb[:])

        def dma_xq(lo, hi):
            return nc.sync.dma_start(
                xq_sb[:, lo:hi, :],
                xq[:, lo * MS:hi * MS].rearrange("p (s m) -> p s m", m=MS))

        def emit_gating():
            ps_gt = psp.tile([E, MS], dt.float32, tag="ps", name="ps_gt")
            for s in range(DS):
                nc.tensor.matmul(
                    ps_gt[:],
                    lhsT=gwt_sb[:, s:s + 1, :],
                    rhs=xf_sb[:, s:s + 1, :],
                    start=(s == 0),
                    stop=(s == DS - 1),
                )
            nc.vector.tensor_scalar_add(lg_sb[:], ps_gt[:], gb_sb[:])

        def emit_softmax():
            for mc in range(MC):
                pst = psp.tile([128, E], dt.float32, tag="ps", name=f"ps_t{mc}")
                nc.tensor.transpose(
                    pst[:], lg_sb[:, mc * 128:(mc + 1) * 128], id8_sb[:]
                )
                mx = small.tile([128, 1], dt.float32, tag="sm1")
                nc.vector.tensor_reduce(
                    mx[:], pst[:], mybir.AxisListType.X, mybir.AluOpType.max
                )
                nmx = small.tile([128, 1], dt.float32, tag="sm1")
                nc.vector.tensor_scalar_mul(nmx[:], mx[:], -1.0)
                ex = small.tile([128, E], dt.float32, tag="sm")
                ssum = small.tile([128, 1], dt.float32, tag="sm1")
                nc.scalar.activation(
                    ex[:], pst[:], mybir.ActivationFunctionType.Exp,
                    bias=nmx[:], scale=1.0, accum_out=ssum[:],
                )
                rcp = small.tile([128, 1], dt.float32, tag="sm1")
                nc.vector.reciprocal(rcp[:], ssum[:])
                nc.vector.tensor_scalar_mul(
                    gate_sb[:, mc * E:(mc + 1) * E], ex[:], rcp[:]
                )

        # ---- Main GEMM + weighted combine ----
        for e in range(E):
            w_sb = wpool.tile([128, DS, H], dt.float8e4, tag="w")
            if e == 0:
                # Startup feed: ONE queue (Sync - fast start, FIFO), in
                # exact consumption order: xq s-slices just ahead of the
                # w0 k-pair slices they multiply against, then the gating
                # inputs, then w1 (emitted next iteration). HBM is shared
                # by all 8 cores, so aggregate delivery (~280-350 GB/s)
                # is the constraint - a second queue just reorders
                # completions and delays the critical piece.
                dma_xq(0, 4)
                for kp in range(KP):
                    if kp in (2, 4, 6):
                        dma_xq(2 * kp, 2 * kp + 4)
                    nc.sync.dma_start(
                        w_sb[:, 2 * kp:2 * kp + 2, :],
                        wq[0:128, kp * 2 * H:(kp + 1) * 2 * H].rearrange(
                            "p (s h) -> p s h", h=H),
                    )
                for j in range(2):
                    nc.sync.dma_start(
                        xf_sb[:, j * 8:(j + 1) * 8, :],
                        xf[:, j * 8 * MS:(j + 1) * 8 * MS].rearrange(
                            "p (s m) -> p s m", m=MS),
                    )
                nc.sync.dma_start(
                    gwt_sb[:], gwt[:, :].rearrange("p (s e) -> p s e", e=E))
            else:
                # Later experts load whole (fewer sem waits, which
                # otherwise split into extra LDWEIGHTS slots).
                nc.sync.dma_start(
                    w_sb[:],
                    wq[e * 128:(e + 1) * 128, :].rearrange(
                        "p (s h) -> p s h", h=H),
                )
            if e == 0:
                # Expert 0 runs as two mc-pair phases (k-major, 8 PSUM
                # banks each) consuming w0 pieces at arrival rate. Raw
                # PSUM is copied to acc on ACT; the gate scale is applied
                # in place after softmax. Gating matmuls sit between the
                # phases where the DMA pressure peaks.
                # Phase (0,1) runs k-major across the mc pair so it can
                # consume w0 pieces at arrival rate; mc 2 and 3 run
                # sequentially afterwards (no DMA dependence) so their
                # first PSUM banks free up before the softmax transposes
                # need the slots.
                for phase_mcs in ((0, 1), (2,), (3,)):
                    pss = {
                        mc: [
                            psp.tile([128, NH], dt.float32, tag="ps",
                                     name=f"ps0_{mc}_{i}")
                            for i in range(HC)
                        ]
                        for mc in phase_mcs
                    }
                    for k in range(KP):
                        for mc in phase_mcs:
                            lhsT = xq_sb[:, 2 * k:2 * k + 2,
                                         mc * 128:(mc + 1) * 128]
                            for hc in range(HC):
                                nc.tensor.matmul(
                                    pss[mc][hc][:],
                                    lhsT=lhsT,
                                    rhs=w_sb[:, 2 * k:2 * k + 2,
                                             hc * NH:(hc + 1) * NH],
                                    start=(k == 0),
                                    stop=(k == KP - 1),
                                    perf_mode=mybir.MatmulPerfMode.DoubleRow,
                                )
                    for mc in phase_mcs:
                        for hc in range(HC):
                            nc.scalar.copy(
                                acc_sb[:, mc * H + hc * NH:
                                       mc * H + (hc + 1) * NH],
                                pss[mc][hc][:],
                            )
                # Gating at the e0/e1 boundary: mc 2's PSUM banks freed
                # early (sequential phases) so the softmax transposes get
                # slots without stalling the PE.
                emit_gating()
                emit_softmax()
                for mc in range(MC):
                    g0_ap = gate_sb[:, mc * E:mc * E + 1]
                    for hc in range(HC):
                        a_ap = acc_sb[:, mc * H + hc * NH:
                                      mc * H + (hc + 1) * NH]
                        nc.scalar.mul(a_ap, a_ap, g0_ap)
                continue

            last = e == E - 1
            for mc in range(MC):
                msl = slice(mc * 128, (mc + 1) * 128)
                pss = [
                    psp.tile([128, NH], dt.float32, tag="ps",
                             name=f"ps_{e}_{mc}_{i}")
                    for i in range(HC)
                ]
                g_ap = gate_sb[:, mc * E + e:mc * E + e + 1]
                if last:
                    # hc-major: each bank completes its 8-matmul k-loop
                    # before the next starts, so combine + out DMA drain
                    # behind the PE instead of after it. The very last
                    # chunk is split in two so the post-matmul tail is a
                    # 256-col combine plus a 128 KB DMA.
                    for hc in range(HC):
                        for k in range(KP):
                            nc.tensor.matmul(
                                pss[hc][:],
                                lhsT=xq_sb[:, 2 * k:2 * k + 2, msl],
                                rhs=w_sb[:, 2 * k:2 * k + 2,
                                         hc * NH:(hc + 1) * NH],
                                start=(k == 0),
                                stop=(k == KP - 1),
                                perf_mode=mybir.MatmulPerfMode.DoubleRow,
                            )
                        final = mc == MC - 1 and hc == HC - 1
                        for q in range(2) if final else range(1):
                            csl = slice(hc * NH + (NH // 2) * q,
                                        hc * NH + (NH // 2) * (q + 1)
                                        ) if final else slice(
                                            hc * NH, (hc + 1) * NH)
                            a_ap = acc_sb[:, mc * H + csl.start:
                                          mc * H + csl.stop]
                            p_ap = pss[hc][:, csl.start - hc * NH:
                                           csl.stop - hc * NH]
                            nc.vector.scalar_tensor_tensor(
                                a_ap, p_ap, g_ap, a_ap,
                                op0=mybir.AluOpType.mult,
                                op1=mybir.AluOpType.add,
                            )
                            eng = nc.scalar if (mc * HC + hc) % 2 == 0 \
                                else nc.sync
                            eng.dma_start(
                                out[mc * 128:(mc + 1) * 128, csl], a_ap)
                else:
                    for k in range(KP):
                        lhsT = xq_sb[:, 2 * k:2 * k + 2, msl]
                        for hc in range(HC):
                            nc.tensor.matmul(
                                pss[hc][:],
                                lhsT=lhsT,
                                rhs=w_sb[:, 2 * k:2 * k + 2,
                                         hc * NH:(hc + 1) * NH],
                                start=(k == 0),
                                stop=(k == KP - 1),
                                perf_mode=mybir.MatmulPerfMode.DoubleRow,
                            )
                    for hc in range(HC):
                        a_ap = acc_sb[:, mc * H + hc * NH:
                                      mc * H + (hc + 1) * NH]
                        nc.vector.scalar_tensor_tensor(
                            a_ap, pss[hc][:], g_ap, a_ap,
                            op0=mybir.AluOpType.mult,
                            op1=mybir.AluOpType.add,
                        )

    nc.compile()
    return nc


def _get_nc() -> bass.Bass:
    global _NC
    if _NC is None:
        _NC = _build_program()
    return _NC


def _sbuf_order(a, cols):
    """[D, cols] -> [128, DS_a * cols] contiguous in (p, s, col) order."""
    d = a.shape[0]
    return np.ascontiguousarray(
        a.reshape(d // 128, 128, cols).transpose(1, 0, 2)
    ).reshape(128, (d // 128) * cols)


def _prep_in_maps(x, gate_w, gate_b, expert_w):
    f8fn = ml_dtypes.float8_e4m3fn
    f8trn = ml_dtypes.float8_e4m3  # same bits as e4m3fn for |v| <= 240

    x = np.asarray(x, dtype=np.float32)
    gate_w = np.asarray(gate_w, dtype=np.float32)
    gate_b = np.asarray(gate_b, dtype=np.float32)
    expert_w = np.asarray(expert_w, dtype=np.float32)

    xT = np.ascontiguousarray(x.T)                       # [D, M] f32
    xT_f16 = xT.astype(np.float16)                       # gating copy
    xqT = xT.astype(f8fn).view(f8trn)                    # [D, M] fp8
    # expert_w [E, H, D] -> per-expert w^T [D, H], quantized, then into
    # SBUF order [E*128, DS*H].
    wqT = np.ascontiguousarray(
        expert_w.transpose(0, 2, 1)
    ).astype(f8fn).view(f8trn)                           # [E, D, H]
    wq_l = np.ascontiguousarray(
        wqT.reshape(E, DS, 128, H).transpose(0, 2, 1, 3)
    ).reshape(E * 128, DS * H)
    gwt_l = _sbuf_order(
        np.ascontiguousarray(gate_w.T).astype(np.float16), E)
    gbb = np.ascontiguousarray(gate_b.reshape(E, 1))

    in_maps = []
    for c in range(NCORES):
        csl = slice(c * MS, (c + 1) * MS)
        in_maps.append({
            "xq": _sbuf_order(np.ascontiguousarray(xqT[:, csl]), MS),
            "xf": _sbuf_order(np.ascontiguousarray(xT_f16[:, csl]), MS),
            "wq": wq_l,
            "gwt": gwt_l,
            "gb": gbb,
        })
    return in_maps


def kernel(x, gate_w, gate_b, expert_w, _trace=False, _trace_kwargs=None):
    nc = _get_nc()
    in_maps = _prep_in_maps(x, gate_w, gate_b, expert_w)
    kw = {}
    if _trace:
        kw["trace"] = True
        kw.update(_trace_kwargs or {})
    res = run_bass_kernel_spmd(nc, in_maps, core_ids=list(range(NCORES)), **kw)
    outp = np.concatenate(
        [np.asarray(res.results[c]["out"]) for c in range(NCORES)], axis=0
    )
    if _trace:
        return outp, res
    return outp
